# revision 4
# baseline (speedup 1.0000x reference)
"""DeepSeekMoE forward on 8 TRN2 NeuronCores.

Strategy (expert-parallel, per the sharding hint):
  - Host computes the (tiny) gate: scores = sqrt(softplus(x @ gate_w)),
    top-2 selection, normalized combine weights, and builds per-expert
    token lists (the "all-to-all dispatch" done host-side since kernel()
    receives full inputs and returns the full output).
  - Core e holds routed expert e's weights and processes the tokens
    routed to it (padded to a common capacity C).
  - The shared expert is split along its intermediate dim I across the
    8 cores (each core computes a 384-wide slice for ALL tokens); the
    partial outputs sum to the exact shared-expert output.
  - Host scatters/sums the per-core outputs back to [B, T, D].

Device compute is bf16 (f32 PSUM accumulation): TRN2 PE does bf16 at
1 cycle/row vs 4 for fp32, and bf16 halves the HBM traffic.
"""

import math

import numpy as np
import ml_dtypes

import concourse.bass as bass
import concourse.tile as tile
from concourse import bacc, mybir
from concourse.bass_utils import run_bass_kernel_spmd

BF16 = np.dtype(ml_dtypes.bfloat16)
DT_BF16 = mybir.dt.bfloat16
DT_F32 = mybir.dt.float32

D = 768            # n_embd
I = 3072           # moe_intermediate_size
E = 8              # n_routed_experts
TOPK = 2
LIMIT = 10.0
NTOK = 2048        # B*T
NCORES = 8
ISH = I // NCORES  # shared-expert I slice per core (384)
DTILES = D // 128  # 6
MI = I // 128      # 24 routed i-tiles
MS = ISH // 128    # 3 shared i-tiles

_BUILD_CACHE: dict = {}
last_results = None  # BassKernelResults of the most recent run (for test.py)


def _chunks(total, step=512):
    out = []
    t0 = 0
    while t0 < total:
        out.append((t0, min(step, total - t0)))
        t0 += step
    return out


def _build(C):
    """Build the SPMD Bass graph for capacity C (tokens per routed expert)."""
    nc = bacc.Bacc("TRN2", target_bir_lowering=False, debug=False)

    ap = lambda name, shape, dt, kind: nc.dram_tensor(name, shape, dt, kind=kind).ap()
    w13 = ap("w13", [2 * MI, 128, DTILES, 128], DT_BF16, "ExternalInput")
    w2 = ap("w2", [128, MI, D], DT_BF16, "ExternalInput")
    w13s = ap("w13s", [2 * MS, 128, DTILES, 128], DT_BF16, "ExternalInput")
    w2s = ap("w2s", [128, MS, D], DT_BF16, "ExternalInput")
    xt = ap("xt", [128, DTILES, NTOK], DT_BF16, "ExternalInput")
    xg = ap("xg", [128, DTILES, C], DT_BF16, "ExternalInput")
    cw = ap("cw", [128, C // 128], DT_F32, "ExternalInput")
    out_r = ap("out_r", [C, D], DT_F32, "ExternalOutput")
    out_s = ap("out_s", [NTOK, D], DT_F32, "ExternalOutput")

    TCR = _chunks(C)      # routed token chunks
    TCS = _chunks(NTOK)   # shared token chunks
    DC = _chunks(D)       # output d chunks (512, 256)

    MIN = mybir.AluOpType.min
    MAX = mybir.AluOpType.max
    SILU = mybir.ActivationFunctionType.Silu
    COPY = mybir.ActivationFunctionType.Copy

    with tile.TileContext(nc) as tc:
        with (
            tc.tile_pool(name="res", bufs=1) as res,
            tc.tile_pool(name="slab", bufs=4) as slabs,
            tc.tile_pool(name="tmp", bufs=4) as tmps,
            tc.tile_pool(name="ev", bufs=4) as evs,
            tc.tile_pool(name="ps", bufs=4, space="PSUM") as ps1,
            tc.tile_pool(name="ps2", bufs=4, space="PSUM") as ps2,
        ):
            xt_sb = res.tile([128, DTILES, NTOK], DT_BF16)
            nc.sync.dma_start(xt_sb[:], xt[:])
            xg_sb = res.tile([128, DTILES, C], DT_BF16)
            nc.sync.dma_start(xg_sb[:], xg[:])
            w2_sb = res.tile([128, MI, D], DT_BF16)
            nc.sync.dma_start(w2_sb[:], w2[:])
            w2s_sb = res.tile([128, MS, D], DT_BF16)
            nc.sync.dma_start(w2s_sb[:], w2s[:])
            cw_sb = res.tile([128, C // 128], DT_F32)
            nc.sync.dma_start(cw_sb[:], cw[:])
            h_sb = res.tile([128, MI, C], DT_BF16)
            hs_sb = res.tile([128, MS, NTOK], DT_BF16)

            def gemm1(npairs, wsrc, x_sb, tchunks, hout):
                # hout[i, t] = silu(min(W1.T x, L)) * clip(W3.T x, -L, L)
                for m in range(npairs):
                    sg = slabs.tile([128, DTILES, 128], DT_BF16, tag="slab")
                    nc.sync.dma_start(sg[:], wsrc[2 * m])
                    su = slabs.tile([128, DTILES, 128], DT_BF16, tag="slab")
                    nc.sync.dma_start(su[:], wsrc[2 * m + 1])
                    for (t0, tl) in tchunks:
                        pg = ps1.tile([128, 512], DT_F32, tag="ps", name="pg")[:, :tl]
                        pu = ps1.tile([128, 512], DT_F32, tag="ps", name="pu")[:, :tl]
                        for d in range(DTILES):
                            nc.tensor.matmul(
                                pg[:], sg[:, d, :], x_sb[:, d, t0:t0 + tl],
                                start=(d == 0), stop=(d == DTILES - 1))
                        for d in range(DTILES):
                            nc.tensor.matmul(
                                pu[:], su[:, d, :], x_sb[:, d, t0:t0 + tl],
                                start=(d == 0), stop=(d == DTILES - 1))
                        tg = tmps.tile([128, 512], DT_F32, tag="tg", name="tg")[:, :tl]
                        nc.vector.tensor_scalar(tg[:], pg[:], LIMIT, None, MIN)
                        sa = tmps.tile([128, 512], DT_F32, tag="sa", name="sa")[:, :tl]
                        nc.scalar.activation(sa[:], tg[:], SILU)
                        tu = tmps.tile([128, 512], DT_F32, tag="tu", name="tu")[:, :tl]
                        nc.vector.tensor_scalar(tu[:], pu[:], LIMIT, -LIMIT, MIN, MAX)
                        nc.vector.tensor_mul(hout[:, m, t0:t0 + tl], sa[:], tu[:])

            def gemm2(nitiles, h, w2sb, ttiles, scale_col, dst):
                # dst[t, d] = (h.T @ w2) * cw[t]
                for tt in range(ttiles):
                    for (d0, dl) in DC:
                        ps = ps2.tile([128, 512], DT_F32, tag="po", name="po")[:, :dl]
                        for m in range(nitiles):
                            nc.tensor.matmul(
                                ps[:], h[:, m, tt * 128:(tt + 1) * 128],
                                w2sb[:, m, d0:d0 + dl],
                                start=(m == 0), stop=(m == nitiles - 1))
                        ev = evs.tile([128, 512], DT_F32, tag="ev", name="ev")[:, :dl]
                        if scale_col is not None:
                            nc.scalar.activation(ev[:], ps[:], COPY,
                                                 scale=scale_col[:, tt:tt + 1])
                        else:
                            nc.scalar.activation(ev[:], ps[:], COPY)
                        nc.sync.dma_start(dst[tt * 128:(tt + 1) * 128, d0:d0 + dl], ev[:])

            gemm1(MI, w13, xg_sb, TCR, h_sb)
            gemm1(MS, w13s, xt_sb, TCS, hs_sb)
            gemm2(MI, h_sb, w2_sb, C // 128, cw_sb, out_r)
            gemm2(MS, hs_sb, w2s_sb, NTOK // 128, None, out_s)

    nc.compile()
    return nc


def _slabify(w):
    """[768, ncols] -> [ncols//128, 128, 6, 128] stationary slabs.

    slab[m, p, a, f] = w[a*128 + p, m*128 + f]
    """
    ncols = w.shape[1]
    return np.ascontiguousarray(
        w.reshape(DTILES, 128, ncols // 128, 128).transpose(2, 1, 0, 3))


def _ptile(a):
    """[R, cols] with R = n*128 -> [128, n, cols] (partition-major)."""
    r, c = a.shape
    return np.ascontiguousarray(a.reshape(r // 128, 128, c).transpose(1, 0, 2))


def kernel(**inputs) -> np.ndarray:
    global last_results
    x = np.asarray(inputs["x"], dtype=np.float32)
    gate_w = np.asarray(inputs["gate_w"], dtype=np.float32)
    gate_bias = np.asarray(inputs["gate_bias"], dtype=np.float32)
    w1 = np.asarray(inputs["w1"], dtype=np.float32)
    w2 = np.asarray(inputs["w2"], dtype=np.float32)
    w3 = np.asarray(inputs["w3"], dtype=np.float32)
    w1s = np.asarray(inputs["w1s"], dtype=np.float32)
    w2s = np.asarray(inputs["w2s"], dtype=np.float32)
    w3s = np.asarray(inputs["w3s"], dtype=np.float32)

    B, T, _ = x.shape
    N = B * T
    assert N == NTOK, f"kernel compiled for {NTOK} tokens, got {N}"
    flat = x.reshape(N, D)

    # ---- gate (host, f32, mirrors reference semantics) ----
    logits = flat @ gate_w                              # [N, E]
    scores = np.sqrt(np.logaddexp(np.float32(0.0), logits)).astype(np.float32)
    routed = scores + gate_bias
    idx = np.argsort(-routed, axis=1, kind="stable")[:, :TOPK]      # [N, K]
    wts = np.take_along_axis(scores, idx, axis=1)
    wts = wts / np.clip(wts.sum(axis=1, keepdims=True), 1e-6, None)

    # ---- dispatch: per-expert token lists ----
    ee = idx.reshape(-1)
    tok = np.repeat(np.arange(N), TOPK)
    ww = wts.reshape(-1).astype(np.float32)
    toks, cwts, counts = [], [], []
    for e in range(E):
        sel = ee == e
        toks.append(tok[sel])
        cwts.append(ww[sel])
        counts.append(int(sel.sum()))
    C = max(128, ((max(counts) + 127) // 128) * 128)

    # ---- per-core input maps ----
    xt_h = _ptile(flat.T.astype(BF16))                  # [128, 6, N]
    in_maps = []
    for e in range(E):
        ce = counts[e]
        xg_full = np.zeros((C, D), dtype=np.float32)
        xg_full[:ce] = flat[toks[e]]
        cw_full = np.zeros(C, dtype=np.float32)
        cw_full[:ce] = cwts[e]

        w13 = np.empty((2 * MI, 128, DTILES, 128), dtype=BF16)
        w13[0::2] = _slabify(w1[e].astype(BF16))
        w13[1::2] = _slabify(w3[e].astype(BF16))
        sl = slice(e * ISH, (e + 1) * ISH)
        w13s = np.empty((2 * MS, 128, DTILES, 128), dtype=BF16)
        w13s[0::2] = _slabify(w1s[:, sl].astype(BF16))
        w13s[1::2] = _slabify(w3s[:, sl].astype(BF16))

        in_maps.append({
            "w13": w13,
            "w2": _ptile(w2[e].astype(BF16)),           # [128, 24, 768]
            "w13s": w13s,
            "w2s": _ptile(w2s[sl].astype(BF16)),        # [128, 3, 768]
            "xt": xt_h,
            "xg": _ptile(xg_full.T.astype(BF16)),       # [128, 6, C]
            "cw": np.ascontiguousarray(
                cw_full.reshape(C // 128, 128).T),      # [128, C//128]
        })

    # ---- build + run ----
    if C not in _BUILD_CACHE:
        _BUILD_CACHE[C] = _build(C)
    nc = _BUILD_CACHE[C]
    last_results = run_bass_kernel_spmd(nc, in_maps, core_ids=list(range(NCORES)))
    res = last_results.results

    # ---- combine (host): sum shared partials, scatter routed outputs ----
    out = res[0]["out_s"].astype(np.float32).copy()
    for c in range(1, NCORES):
        out += res[c]["out_s"]
    for e in range(E):
        ce = counts[e]
        if ce:
            out[toks[e]] += res[e]["out_r"][:ce]
    return out.reshape(B, T, D).astype(np.float32)


# revision 6
# speedup vs baseline: 1.0507x; 1.0507x over previous
"""DeepSeekMoE forward on 8 TRN2 NeuronCores.

Strategy (expert-parallel, per the sharding hint):
  - Host computes the (tiny) gate: scores = sqrt(softplus(x @ gate_w)),
    top-2 selection, normalized combine weights, and builds per-expert
    token lists (the "all-to-all dispatch" done host-side since kernel()
    receives full inputs and returns the full output).
  - Core e holds routed expert e's weights and processes the tokens
    routed to it (padded to a common capacity C).
  - The shared expert is split along its intermediate dim I across the
    8 cores (each core computes a 384-wide slice for ALL tokens); the
    partial outputs sum to the exact shared-expert output.
  - Host scatters/sums the per-core outputs back to [B, T, D].

Device compute is bf16 (f32 PSUM accumulation): TRN2 PE does bf16 at
1 cycle/row vs 4 for fp32, and bf16 halves the HBM traffic.
"""

import math

import numpy as np
import ml_dtypes

import concourse.bass as bass
import concourse.tile as tile
from concourse import bacc, mybir
from concourse.bass_utils import run_bass_kernel_spmd

BF16 = np.dtype(ml_dtypes.bfloat16)
DT_BF16 = mybir.dt.bfloat16
DT_F32 = mybir.dt.float32

D = 768            # n_embd
I = 3072           # moe_intermediate_size
E = 8              # n_routed_experts
TOPK = 2
LIMIT = 10.0
NTOK = 2048        # B*T
NCORES = 8
ISH = I // NCORES  # shared-expert I slice per core (384)
DTILES = D // 128  # 6
MI = I // 128      # 24 routed i-tiles
MS = ISH // 128    # 3 shared i-tiles

_BUILD_CACHE: dict = {}
last_results = None  # BassKernelResults of the most recent run (for test.py)


def _chunks(total, step=512):
    out = []
    t0 = 0
    while t0 < total:
        out.append((t0, min(step, total - t0)))
        t0 += step
    return out


def _build(C):
    """Build the SPMD Bass graph for capacity C (tokens per routed expert)."""
    nc = bacc.Bacc("TRN2", target_bir_lowering=False, debug=False)

    ap = lambda name, shape, dt, kind: nc.dram_tensor(name, shape, dt, kind=kind).ap()
    w13 = ap("w13", [2 * MI, 128, DTILES, 128], DT_BF16, "ExternalInput")
    w2 = ap("w2", [128, MI, D], DT_BF16, "ExternalInput")
    w13s = ap("w13s", [2 * MS, 128, DTILES, 128], DT_BF16, "ExternalInput")
    w2s = ap("w2s", [128, MS, D], DT_BF16, "ExternalInput")
    xt = ap("xt", [128, DTILES, NTOK], DT_BF16, "ExternalInput")
    xg = ap("xg", [128, DTILES, C], DT_BF16, "ExternalInput")
    cw = ap("cw", [128, C // 128], DT_F32, "ExternalInput")
    out_r = ap("out_r", [C, D], DT_F32, "ExternalOutput")
    out_s = ap("out_s", [NTOK, D], DT_F32, "ExternalOutput")

    TCR = _chunks(C)      # routed token chunks
    TCS = _chunks(NTOK)   # shared token chunks
    DC = _chunks(D)       # output d chunks (512, 256)

    MIN = mybir.AluOpType.min
    MAX = mybir.AluOpType.max
    SILU = mybir.ActivationFunctionType.Silu
    COPY = mybir.ActivationFunctionType.Copy

    with tile.TileContext(nc) as tc:
        with (
            tc.tile_pool(name="res", bufs=1) as res,
            tc.tile_pool(name="slab", bufs=6) as slabs,
            tc.tile_pool(name="tmp", bufs=4) as tmps,
            tc.tile_pool(name="ev", bufs=4) as evs,
            tc.tile_pool(name="ps", bufs=8, space="PSUM") as ps1,
        ):
            ps2 = ps1
            # xg first: it gates the very first matmul. The big resident
            # tensors (xt, w2, ...) are needed only later — issue them on
            # the gpsimd DMA queue so they don't delay the slab stream.
            xg_sb = res.tile([128, DTILES, C], DT_BF16)
            nc.sync.dma_start(xg_sb[:], xg[:])
            xt_sb = res.tile([128, DTILES, NTOK], DT_BF16)
            nc.gpsimd.dma_start(xt_sb[:], xt[:])
            w2_sb = res.tile([128, MI, D], DT_BF16)
            nc.gpsimd.dma_start(w2_sb[:], w2[:])
            w2s_sb = res.tile([128, MS, D], DT_BF16)
            nc.gpsimd.dma_start(w2s_sb[:], w2s[:])
            cw_sb = res.tile([128, C // 128], DT_F32)
            nc.gpsimd.dma_start(cw_sb[:], cw[:])
            h_sb = res.tile([128, MI, C], DT_BF16)
            hs_sb = res.tile([128, MS, NTOK], DT_BF16)

            def gemm1(npairs, wsrc, x_sb, tchunks, hout):
                # hout[i, t] = silu(min(W1.T x, L)) * clip(W3.T x, -L, L)
                for m in range(npairs):
                    sg = slabs.tile([128, DTILES, 128], DT_BF16, tag="slab")
                    nc.sync.dma_start(sg[:], wsrc[2 * m])
                    su = slabs.tile([128, DTILES, 128], DT_BF16, tag="slab")
                    nc.sync.dma_start(su[:], wsrc[2 * m + 1])
                    for (t0, tl) in tchunks:
                        pg = ps1.tile([128, 512], DT_F32, tag="ps", name="pg")[:, :tl]
                        pu = ps1.tile([128, 512], DT_F32, tag="ps", name="pu")[:, :tl]
                        for d in range(DTILES):
                            nc.tensor.matmul(
                                pg[:], sg[:, d, :], x_sb[:, d, t0:t0 + tl],
                                start=(d == 0), stop=(d == DTILES - 1))
                        for d in range(DTILES):
                            nc.tensor.matmul(
                                pu[:], su[:, d, :], x_sb[:, d, t0:t0 + tl],
                                start=(d == 0), stop=(d == DTILES - 1))
                        tg = tmps.tile([128, 512], DT_F32, tag="tg", name="tg")[:, :tl]
                        nc.vector.tensor_scalar(tg[:], pg[:], LIMIT, None, MIN)
                        sa = tmps.tile([128, 512], DT_F32, tag="sa", name="sa")[:, :tl]
                        nc.scalar.activation(sa[:], tg[:], SILU)
                        tu = tmps.tile([128, 512], DT_F32, tag="tu", name="tu")[:, :tl]
                        nc.vector.tensor_scalar(tu[:], pu[:], LIMIT, -LIMIT, MIN, MAX)
                        nc.vector.tensor_mul(hout[:, m, t0:t0 + tl], sa[:], tu[:])

            def gemm2(nitiles, h, w2sb, ttiles, scale_col, dst):
                # dst[t, d] = (h.T @ w2) * cw[t]
                for tt in range(ttiles):
                    for (d0, dl) in DC:
                        ps = ps2.tile([128, 512], DT_F32, tag="ps", name="po")[:, :dl]
                        for m in range(nitiles):
                            nc.tensor.matmul(
                                ps[:], h[:, m, tt * 128:(tt + 1) * 128],
                                w2sb[:, m, d0:d0 + dl],
                                start=(m == 0), stop=(m == nitiles - 1))
                        ev = evs.tile([128, 512], DT_F32, tag="ev", name="ev")[:, :dl]
                        if scale_col is not None:
                            nc.scalar.activation(ev[:], ps[:], COPY,
                                                 scale=scale_col[:, tt:tt + 1])
                        else:
                            nc.scalar.activation(ev[:], ps[:], COPY)
                        nc.sync.dma_start(dst[tt * 128:(tt + 1) * 128, d0:d0 + dl], ev[:])

            gemm1(MI, w13, xg_sb, TCR, h_sb)
            gemm1(MS, w13s, xt_sb, TCS, hs_sb)
            gemm2(MI, h_sb, w2_sb, C // 128, cw_sb, out_r)
            gemm2(MS, hs_sb, w2s_sb, NTOK // 128, None, out_s)

    nc.compile()
    return nc


def _slabify(w):
    """[768, ncols] -> [ncols//128, 128, 6, 128] stationary slabs.

    slab[m, p, a, f] = w[a*128 + p, m*128 + f]
    """
    ncols = w.shape[1]
    return np.ascontiguousarray(
        w.reshape(DTILES, 128, ncols // 128, 128).transpose(2, 1, 0, 3))


def _ptile(a):
    """[R, cols] with R = n*128 -> [128, n, cols] (partition-major)."""
    r, c = a.shape
    return np.ascontiguousarray(a.reshape(r // 128, 128, c).transpose(1, 0, 2))


def kernel(**inputs) -> np.ndarray:
    global last_results
    x = np.asarray(inputs["x"], dtype=np.float32)
    gate_w = np.asarray(inputs["gate_w"], dtype=np.float32)
    gate_bias = np.asarray(inputs["gate_bias"], dtype=np.float32)
    w1 = np.asarray(inputs["w1"], dtype=np.float32)
    w2 = np.asarray(inputs["w2"], dtype=np.float32)
    w3 = np.asarray(inputs["w3"], dtype=np.float32)
    w1s = np.asarray(inputs["w1s"], dtype=np.float32)
    w2s = np.asarray(inputs["w2s"], dtype=np.float32)
    w3s = np.asarray(inputs["w3s"], dtype=np.float32)

    B, T, _ = x.shape
    N = B * T
    assert N == NTOK, f"kernel compiled for {NTOK} tokens, got {N}"
    flat = x.reshape(N, D)

    # ---- gate (host, f32, mirrors reference semantics) ----
    logits = flat @ gate_w                              # [N, E]
    scores = np.sqrt(np.logaddexp(np.float32(0.0), logits)).astype(np.float32)
    routed = scores + gate_bias
    idx = np.argsort(-routed, axis=1, kind="stable")[:, :TOPK]      # [N, K]
    wts = np.take_along_axis(scores, idx, axis=1)
    wts = wts / np.clip(wts.sum(axis=1, keepdims=True), 1e-6, None)

    # ---- dispatch: per-expert token lists ----
    ee = idx.reshape(-1)
    tok = np.repeat(np.arange(N), TOPK)
    ww = wts.reshape(-1).astype(np.float32)
    toks, cwts, counts = [], [], []
    for e in range(E):
        sel = ee == e
        toks.append(tok[sel])
        cwts.append(ww[sel])
        counts.append(int(sel.sum()))
    C = max(128, ((max(counts) + 127) // 128) * 128)

    # ---- per-core input maps ----
    xt_h = _ptile(flat.T.astype(BF16))                  # [128, 6, N]
    in_maps = []
    for e in range(E):
        ce = counts[e]
        xg_full = np.zeros((C, D), dtype=np.float32)
        xg_full[:ce] = flat[toks[e]]
        cw_full = np.zeros(C, dtype=np.float32)
        cw_full[:ce] = cwts[e]

        w13 = np.empty((2 * MI, 128, DTILES, 128), dtype=BF16)
        w13[0::2] = _slabify(w1[e].astype(BF16))
        w13[1::2] = _slabify(w3[e].astype(BF16))
        sl = slice(e * ISH, (e + 1) * ISH)
        w13s = np.empty((2 * MS, 128, DTILES, 128), dtype=BF16)
        w13s[0::2] = _slabify(w1s[:, sl].astype(BF16))
        w13s[1::2] = _slabify(w3s[:, sl].astype(BF16))

        in_maps.append({
            "w13": w13,
            "w2": _ptile(w2[e].astype(BF16)),           # [128, 24, 768]
            "w13s": w13s,
            "w2s": _ptile(w2s[sl].astype(BF16)),        # [128, 3, 768]
            "xt": xt_h,
            "xg": _ptile(xg_full.T.astype(BF16)),       # [128, 6, C]
            "cw": np.ascontiguousarray(
                cw_full.reshape(C // 128, 128).T),      # [128, C//128]
        })

    # ---- build + run ----
    if C not in _BUILD_CACHE:
        _BUILD_CACHE[C] = _build(C)
    nc = _BUILD_CACHE[C]
    last_results = run_bass_kernel_spmd(nc, in_maps, core_ids=list(range(NCORES)))
    res = last_results.results

    # ---- combine (host): sum shared partials, scatter routed outputs ----
    out = res[0]["out_s"].astype(np.float32).copy()
    for c in range(1, NCORES):
        out += res[c]["out_s"]
    for e in range(E):
        ce = counts[e]
        if ce:
            out[toks[e]] += res[e]["out_r"][:ce]
    return out.reshape(B, T, D).astype(np.float32)
